# revision 5
# baseline (speedup 1.0000x reference)
"""Multi-head attention kernel for Trainium2, tensor-parallel over heads on 8 cores.

Strategy (per core c, heads [2c, 2c+1]):
  - host feeds X^T [D, B*S] (shared), per-core transposed head weights, and the
    matching Wo column-slice; each core computes a full-shape partial of the
    output projection (fp16), host sums the 8 partials and adds bo.
  - on device everything is computed in "transposed" orientation so every
    matmul contracts over the partition dim (only V needs a PE-transpose).
    All matmul operands are fp16, accumulation stays fp32:
      QT/KT/VT [e, s] = W @ X^T          (fp16 matmuls, N=512)
      S^T [t, s]      = KT.T @ QT        (per (b, head), C=64, head pair
                                          row-group-packed and concurrent)
      P^T             = exp(S^T / 8)     (ACT, PSUM->SBUF; the wall: ~284us)
      [avT ; l]       = [V | 1].T @ P^T  (fused unnormalized attention + sum)
      bc              = 1s.T @ (1/l)     (C=1 PE broadcast matmul, fp32r)
      Z               = avT * bc         (DVE)
      out_partial     = Z.T @ WoT_slice  (PSUM -> SBUF fp16 -> DRAM)

  Scheduling: the exp stream on ACT (~1.1us per t-chunk) is the bottleneck;
  everything else is emitted so the Tile scheduler treats it as PE/DVE
  gap-filler. Batch b+1's projections are emitted AFTER attn(b) (priority =
  emission order), out-projection of s-tile n is deferred into s-tile n+1's
  t-loop, and batch 0 feeds K first so the exp stream starts ASAP.
"""

import numpy as np

import concourse.bass as bass
import concourse.mybir as mybir
import concourse.tile as tile
from concourse import bacc
from concourse.bass_utils import run_bass_kernel_spmd
from concourse.masks import make_identity

# Problem shapes (hardcoded per contract).
B, S, D = 4, 2048, 1024
H, E = 16, 64
NCORES = 8
HPC = H // NCORES          # heads per core = 2
EC = HPC * E               # per-core head width = 128
BS = B * S                 # 8192 rows
P = 128
DC = D // P                # 8 contraction chunks for the projections
ST = 512                   # s tile (matmul moving free dim)
N_ST = S // ST             # 4 s-tiles per batch
TCH = S // P               # 16 key chunks per batch

F32 = mybir.dt.float32
F32R = mybir.dt.float32r
F16 = mybir.dt.float16
EXP = mybir.ActivationFunctionType.Exp


def _r(ap):
    return ap.bitcast(F32R)


def build_module():
    """Build the single-core Bass module (same NEFF runs SPMD on all 8 cores)."""
    from contextlib import ExitStack

    nc = bacc.Bacc("TRN2", target_bir_lowering=False, debug=False)
    xt = nc.dram_tensor("xt", [D, BS], F16, kind="ExternalInput").ap()
    wq = nc.dram_tensor("wq_t", [D, EC], F16, kind="ExternalInput").ap()
    wk = nc.dram_tensor("wk_t", [D, EC], F16, kind="ExternalInput").ap()
    wv = nc.dram_tensor("wv_t", [D, EC], F16, kind="ExternalInput").ap()
    bq = nc.dram_tensor("bq", [EC, 1], F32, kind="ExternalInput").ap()
    bk = nc.dram_tensor("bk", [EC, 1], F32, kind="ExternalInput").ap()
    bv = nc.dram_tensor("bv", [EC, 1], F32, kind="ExternalInput").ap()
    wo = nc.dram_tensor("wo_t", [EC, D], F16, kind="ExternalInput").ap()
    outp = nc.dram_tensor("out_p", [BS, D], F16, kind="ExternalOutput").ap()

    xt_r = xt.rearrange("(dc p) s -> p dc s", p=P)    # [128, 8, 8192]
    wq_r = wq.rearrange("(dc p) e -> p dc e", p=P)    # [128, 8, 128]
    wk_r = wk.rearrange("(dc p) e -> p dc e", p=P)
    wv_r = wv.rearrange("(dc p) e -> p dc e", p=P)

    with tile.TileContext(nc) as tc, ExitStack() as ctx:
        singles = ctx.enter_context(tc.tile_pool(name="singles", bufs=1))

        wq_sb = singles.tile([P, DC, EC], F16, tag="wq")
        wk_sb = singles.tile([P, DC, EC], F16, tag="wk")
        wv_sb = singles.tile([P, DC, EC], F16, tag="wv")
        nc.sync.dma_start(wq_sb[:], wq_r)
        nc.sync.dma_start(wk_sb[:], wk_r)
        nc.sync.dma_start(wv_sb[:], wv_r)
        bq_sb = singles.tile([EC, 1], F32, tag="bq")
        bk_sb = singles.tile([EC, 1], F32, tag="bk")
        bv_sb = singles.tile([EC, 1], F32, tag="bv")
        nc.sync.dma_start(bq_sb[:], bq)
        nc.sync.dma_start(bk_sb[:], bk)
        nc.sync.dma_start(bv_sb[:], bv)
        wo_sb = singles.tile([EC, D], F16, tag="wo")
        nc.sync.dma_start(wo_sb[:], wo)
        ident = singles.tile([P, P], F16, tag="ident")
        make_identity(nc, ident[:])
        ones16 = singles.tile([1, E], F16, tag="ones16")
        nc.vector.memset(ones16[:], 1.0)

        # Per-batch persistent activations: [e, s] projections and V_ext.
        qt = [singles.tile([EC, S], F16, tag=f"qt{b}", name=f"qt{b}") for b in range(B)]
        kt = [singles.tile([EC, S], F16, tag=f"kt{b}", name=f"kt{b}") for b in range(B)]
        vt = [singles.tile([EC, S], F16, tag=f"vtz{b}", name=f"vt{b}") for b in range(B)]
        # V_ext layout: [t-part, t-chunk, 130] = [V_h0 | 1 | V_h1 | 1]
        vx = [singles.tile([P, TCH, 2 * E + 2], F16, tag=f"vx{b}", name=f"vx{b}") for b in range(B)]
        for b in range(B):
            nc.vector.memset(vx[b][:, :, E : E + 1], 1.0)
            nc.vector.memset(vx[b][:, :, 2 * E + 1 : 2 * E + 2], 1.0)

        z = [singles.tile([EC, S], F16, tag=f"z{b}", name=f"z{b}") for b in range(B)]
        xts = [[None] * N_ST for _ in range(B)]

        with (
            tc.tile_pool(name="xload", bufs=6) as xpool,
            tc.tile_pool(name="pexp", bufs=8) as ppool,
            tc.tile_pool(name="bcast", bufs=3) as bpool,
            tc.tile_pool(name="ostage", bufs=4) as opool,
            tc.tile_pool(name="psum", bufs=2, space="PSUM") as psum,
            tc.tile_pool(name="psum_av", bufs=1, space="PSUM") as psum_av,
        ):

            def emit_x(b):
                for st in range(N_ST):
                    g = b * N_ST + st
                    x_t = xpool.tile([P, DC, ST], F16, tag="xt", name="x_t")
                    nc.sync.dma_start(x_t[:], xt_r[:, :, g * ST : (g + 1) * ST])
                    xts[b][st] = x_t

            def emit_proj(b, st, w_sb, b_sb, dst):
                # One 512-wide projection chain: dst[:, st] = W @ X^T + bias
                sl = slice(st * ST, (st + 1) * ST)
                ps = psum.tile([P, ST], F32, tag="mm", name="ps")
                for dc in range(DC):
                    nc.tensor.matmul(
                        ps[:], w_sb[:, dc], xts[b][st][:, dc],
                        start=(dc == 0), stop=(dc == DC - 1),
                    )
                nc.vector.tensor_scalar_add(dst[:, sl], ps[:], b_sb[:])

            def emit_trans(b, chunks):
                # PE-transpose batch b's V chunks into V_ext.
                for tch in chunks:
                    tp = psum.tile([P, ST], F16, tag="mm", name="tp")
                    nc.tensor.transpose(
                        tp[:, 0:P], vt[b][:, tch * P : (tch + 1) * P], ident[:]
                    )
                    nc.vector.tensor_copy(vx[b][:, tch, 0:E], tp[:, 0:E])
                    nc.vector.tensor_copy(
                        vx[b][:, tch, E + 1 : 2 * E + 1], tp[:, E : 2 * E]
                    )

            def emit_outproj(b, st):
                for c in range(ST // P):
                    zsl = slice(st * ST + c * P, st * ST + (c + 1) * P)
                    rows = slice(b * S + st * ST + c * P, b * S + st * ST + (c + 1) * P)
                    for oh in range(D // 512):
                        po = psum.tile([P, ST], F32, tag="mm", name="po")
                        nc.tensor.matmul(
                            po[:], z[b][:, zsl], wo_sb[:, oh * 512 : (oh + 1) * 512],
                            start=True, stop=True,
                        )
                        osb = opool.tile([P, 512], F16, tag="osb", name="osb")
                        nc.vector.tensor_copy(osb[:], po[:])
                        nc.sync.dma_start(outp[rows, oh * 512 : (oh + 1) * 512], osb[:])

            pending = [None]

            def emit_attn(b, st, hooks=None):
                ssl = slice(st * ST, (st + 1) * ST)
                av = psum_av.tile([P, 2, ST], F32, tag="av", name="av")
                av0 = av[:, 0]
                av1 = av[:, 1]
                for t in range(TCH):
                    tsl = slice(t * P, (t + 1) * P)
                    sc = psum.tile([P, 2, ST], F32, tag="sc", name="sc")
                    nc.tensor.matmul(
                        sc[:, 0], kt[b][0:E, tsl], qt[b][0:E, ssl],
                        start=True, stop=True,
                    )
                    nc.tensor.matmul(
                        sc[:, 1], kt[b][E : 2 * E, tsl], qt[b][E : 2 * E, ssl],
                        start=True, stop=True,
                    )
                    pt = ppool.tile([P, 2, ST], F16, tag="pt", name="pt")
                    nc.scalar.activation(pt[:], sc[:], EXP, scale=0.125)
                    if hooks is not None and t in hooks:
                        hooks[t]()
                    if t == 8 and pending[0] is not None:
                        emit_outproj(*pending[0])
                        pending[0] = None
                    nc.tensor.matmul(
                        av0[0 : E + 1], vx[b][:, t, 0 : E + 1], pt[:, 0],
                        start=(t == 0), stop=(t == TCH - 1),
                    )
                    nc.tensor.matmul(
                        av1[0 : E + 1], vx[b][:, t, E + 1 : 2 * E + 2], pt[:, 1],
                        start=(t == 0), stop=(t == TCH - 1),
                    )
                # Drain: unnormalized copy out of PSUM (frees the av bank),
                # reciprocal of the fused row-sums, partition-broadcast via a
                # C=1 PE matmul (1s.T @ lrecip), then normalize.
                with tc.high_priority():
                    nc.vector.tensor_copy(z[b][0:E, ssl], av0[0:E])
                    nc.vector.tensor_copy(z[b][E : 2 * E, ssl], av1[0:E])
                    lr = bpool.tile([1, 2, ST], F32, tag="lr", name="lr")
                    nc.vector.tensor_copy(lr[0:1], av[E : E + 1, :, :])
                    nc.vector.reciprocal_approx_fast(out=lr[0:1], in_=lr[0:1])
                    lr16 = bpool.tile([1, 2, ST], F16, tag="lr16", name="lr16")
                    nc.vector.tensor_copy(lr16[0:1], lr[0:1])
                bc0 = psum.tile([E, ST], F32, tag="mm", name="bc0")
                nc.tensor.matmul(
                    bc0[:], ones16[0:1, :], lr16[0:1, 0, :],
                    start=True, stop=True,
                )
                nc.vector.tensor_mul(z[b][0:E, ssl], z[b][0:E, ssl], bc0[:])
                bc1 = psum.tile([E, ST], F32, tag="mm", name="bc1")
                nc.tensor.matmul(
                    bc1[:], ones16[0:1, :], lr16[0:1, 1, :],
                    start=True, stop=True,
                )
                nc.vector.tensor_mul(z[b][E : 2 * E, ssl], z[b][E : 2 * E, ssl], bc1[:])
                pending[0] = (b, st)

            # ---- batch 0: K first so the exp stream starts ASAP ----
            emit_x(0)
            emit_proj(0, 0, wk_sb, bk_sb, kt[0])
            emit_proj(0, 1, wk_sb, bk_sb, kt[0])
            emit_proj(0, 0, wq_sb, bq_sb, qt[0])
            emit_proj(0, 0, wv_sb, bv_sb, vt[0])

            def h(*fns):
                def run():
                    for f in fns:
                        f()
                return run

            b0_hooks = {
                0: h(lambda: emit_trans(0, range(0, 4))),
                1: h(lambda: emit_proj(0, 2, wk_sb, bk_sb, kt[0])),
                3: h(lambda: emit_proj(0, 1, wv_sb, bv_sb, vt[0]),
                     lambda: emit_trans(0, range(4, 8))),
                5: h(lambda: emit_proj(0, 3, wk_sb, bk_sb, kt[0])),
                7: h(lambda: emit_proj(0, 2, wv_sb, bv_sb, vt[0]),
                     lambda: emit_trans(0, range(8, 12))),
                11: h(lambda: emit_proj(0, 3, wv_sb, bv_sb, vt[0]),
                      lambda: emit_trans(0, range(12, 16))),
            }
            for st in range(N_ST):
                emit_attn(0, st, hooks=b0_hooks if st == 0 else None)
                if st + 1 < N_ST:
                    emit_proj(0, st + 1, wq_sb, bq_sb, qt[0])

            # ---- batches 1..3: projections emitted after attn(b-1) so they
            # fill PE gaps under the previous batch's exp stream ----
            for b in range(1, B):
                emit_x(b)
                for st in range(N_ST):
                    emit_proj(b, st, wk_sb, bk_sb, kt[b])
                for st in range(N_ST):
                    emit_proj(b, st, wq_sb, bq_sb, qt[b])
                for st in range(N_ST):
                    emit_proj(b, st, wv_sb, bv_sb, vt[b])
                emit_trans(b, range(TCH))
                for st in range(N_ST):
                    emit_attn(b, st)

            emit_outproj(*pending[0])
    nc.finalize()
    return nc


_NC_CACHE = None


def _get_module():
    global _NC_CACHE
    if _NC_CACHE is None:
        _NC_CACHE = build_module()
    return _NC_CACHE


def prepare_in_maps(inputs):
    x = np.ascontiguousarray(np.asarray(inputs["input_matrix"], np.float32))
    wq = np.asarray(inputs["Wq"], np.float32)
    wk = np.asarray(inputs["Wk"], np.float32)
    wv = np.asarray(inputs["Wv"], np.float32)
    bq = np.asarray(inputs["bq"], np.float32)
    bk = np.asarray(inputs["bk"], np.float32)
    bv = np.asarray(inputs["bv"], np.float32)
    wo = np.asarray(inputs["Wo"], np.float32)

    xt = np.ascontiguousarray(x.reshape(BS, D).T.astype(np.float16))  # [D, BS]
    in_maps = []
    for c in range(NCORES):
        hs = slice(HPC * c, HPC * (c + 1))
        m = {
            "xt": xt,
            "wq_t": np.ascontiguousarray(wq[hs].transpose(2, 0, 1).reshape(D, EC).astype(np.float16)),
            "wk_t": np.ascontiguousarray(wk[hs].transpose(2, 0, 1).reshape(D, EC).astype(np.float16)),
            "wv_t": np.ascontiguousarray(wv[hs].transpose(2, 0, 1).reshape(D, EC).astype(np.float16)),
            "bq": np.ascontiguousarray(bq[hs].reshape(EC, 1)),
            "bk": np.ascontiguousarray(bk[hs].reshape(EC, 1)),
            "bv": np.ascontiguousarray(bv[hs].reshape(EC, 1)),
            "wo_t": np.ascontiguousarray(wo[:, EC * c : EC * (c + 1)].T.astype(np.float16)),
        }
        in_maps.append(m)
    return in_maps


def finish(results, inputs):
    bo = np.asarray(inputs["bo"], np.float32)
    acc = results[0]["out_p"].astype(np.float32)
    for r in results[1:]:
        acc += r["out_p"].astype(np.float32)
    out = (acc + bo).astype(np.float32)
    return out.reshape(B, S, D)


def kernel(**inputs):
    nc = _get_module()
    in_maps = prepare_in_maps(inputs)
    res = run_bass_kernel_spmd(nc, in_maps, core_ids=list(range(NCORES)))
    return finish(res.results, inputs)


if __name__ == "__main__":
    import reference

    inputs = {k: np.asarray(v) for k, v in reference.setup_inputs().items()}
    out = kernel(**inputs)
    print(out.shape, out.dtype)


# revision 10
# speedup vs baseline: 1.1438x; 1.1438x over previous
"""Multi-head attention kernel for Trainium2, tensor-parallel over heads on 8 cores.

Strategy (per core c, heads [2c, 2c+1]):
  - host feeds X^T [D, B*S] (shared), per-core transposed head weights, and the
    matching Wo column-slice; each core computes a full-shape partial of the
    output projection (fp16), host sums the 8 partials and adds bo.
  - on device everything is computed in "transposed" orientation so every
    matmul contracts over the partition dim (only V needs a PE-transpose).
    All matmul operands are fp16, accumulation stays fp32:
      QT/KT/VT [e, s] = W @ X^T          (fp16 matmuls, N=512)
      S^T [t, s]      = KT.T @ QT        (per (b, head), C=64, head pair
                                          row-group-packed and concurrent)
      P^T             = exp(S^T / 8)     (ACT, PSUM->SBUF; the wall: ~284us)
      [avT ; l]       = [V | 1].T @ P^T  (fused unnormalized attention + sum)
      bc              = 1s.T @ (1/l)     (C=1 PE broadcast matmul, fp32r)
      Z               = avT * bc         (DVE)
      out_partial     = Z.T @ WoT_slice  (PSUM -> SBUF fp16 -> DRAM)

  Scheduling: the exp stream on ACT (~1.1us per t-chunk) is the bottleneck;
  everything else is emitted so the Tile scheduler treats it as PE/DVE
  gap-filler. Batch b+1's projections are emitted AFTER attn(b) (priority =
  emission order), out-projection of s-tile n is deferred into s-tile n+1's
  t-loop, and batch 0 feeds K first so the exp stream starts ASAP.
"""

import numpy as np

import concourse.bass as bass
import concourse.mybir as mybir
import concourse.tile as tile
from concourse import bacc
from concourse.bass_utils import run_bass_kernel_spmd
from concourse.masks import make_identity

# Problem shapes (hardcoded per contract).
B, S, D = 4, 2048, 1024
H, E = 16, 64
NCORES = 8
HPC = H // NCORES          # heads per core = 2
EC = HPC * E               # per-core head width = 128
BS = B * S                 # 8192 rows
P = 128
DC = D // P                # 8 contraction chunks for the projections
ST = 512                   # s tile (matmul moving free dim)
N_ST = S // ST             # 4 s-tiles per batch
TCH = S // P               # 16 key chunks per batch

F32 = mybir.dt.float32
F32R = mybir.dt.float32r
F16 = mybir.dt.float16
EXP = mybir.ActivationFunctionType.Exp


def _r(ap):
    return ap.bitcast(F32R)


def build_module():
    """Build the single-core Bass module (same NEFF runs SPMD on all 8 cores)."""
    from contextlib import ExitStack

    nc = bacc.Bacc("TRN2", target_bir_lowering=False, debug=False)
    xt = nc.dram_tensor("xt", [D, BS], F16, kind="ExternalInput").ap()
    wq = nc.dram_tensor("wq_t", [D, EC], F16, kind="ExternalInput").ap()
    wk = nc.dram_tensor("wk_t", [D, EC], F16, kind="ExternalInput").ap()
    wv = nc.dram_tensor("wv_t", [D, EC], F16, kind="ExternalInput").ap()
    bq = nc.dram_tensor("bq", [EC, 1], F32, kind="ExternalInput").ap()
    bk = nc.dram_tensor("bk", [EC, 1], F32, kind="ExternalInput").ap()
    bv = nc.dram_tensor("bv", [EC, 1], F32, kind="ExternalInput").ap()
    wo = nc.dram_tensor("wo_t", [EC, D], F16, kind="ExternalInput").ap()
    outp = nc.dram_tensor("out_p", [BS, D], F16, kind="ExternalOutput").ap()

    xt_r = xt.rearrange("(dc p) s -> p dc s", p=P)    # [128, 8, 8192]
    wq_r = wq.rearrange("(dc p) e -> p dc e", p=P)    # [128, 8, 128]
    wk_r = wk.rearrange("(dc p) e -> p dc e", p=P)
    wv_r = wv.rearrange("(dc p) e -> p dc e", p=P)

    with tile.TileContext(nc) as tc, ExitStack() as ctx:
        singles = ctx.enter_context(tc.tile_pool(name="singles", bufs=1))

        wq_sb = singles.tile([P, DC, EC], F16, tag="wq")
        wk_sb = singles.tile([P, DC, EC], F16, tag="wk")
        wv_sb = singles.tile([P, DC, EC], F16, tag="wv")
        nc.sync.dma_start(wq_sb[:], wq_r)
        nc.sync.dma_start(wk_sb[:], wk_r)
        nc.sync.dma_start(wv_sb[:], wv_r)
        bq_sb = singles.tile([EC, 1], F32, tag="bq")
        bk_sb = singles.tile([EC, 1], F32, tag="bk")
        bv_sb = singles.tile([EC, 1], F32, tag="bv")
        nc.sync.dma_start(bq_sb[:], bq)
        nc.sync.dma_start(bk_sb[:], bk)
        nc.sync.dma_start(bv_sb[:], bv)
        wo_sb = singles.tile([EC, D], F16, tag="wo")
        nc.sync.dma_start(wo_sb[:], wo)
        ident = singles.tile([P, P], F16, tag="ident")
        make_identity(nc, ident[:])


        # Per-batch persistent activations: [e, s] projections and V_ext.
        qt = [singles.tile([EC, S], F16, tag=f"qt{b}", name=f"qt{b}") for b in range(B)]
        kt = [singles.tile([EC, S], F16, tag=f"kt{b}", name=f"kt{b}") for b in range(B)]
        vt = [singles.tile([EC, S], F16, tag=f"vtz{b}", name=f"vt{b}") for b in range(B)]
        # V_ext layout: [t-part, t-chunk, 130] = [V_h0 | 1 | V_h1 | 1]
        vx = [singles.tile([P, TCH, 2 * E + 2], F16, tag=f"vx{b}", name=f"vx{b}") for b in range(B)]
        for b in range(B):
            nc.vector.memset(vx[b][:, :, E : E + 1], 1.0)
            nc.vector.memset(vx[b][:, :, 2 * E + 1 : 2 * E + 2], 1.0)

        z = [singles.tile([EC, S], F16, tag=f"z{b}", name=f"z{b}") for b in range(B)]
        xts = [[None] * N_ST for _ in range(B)]

        with (
            tc.tile_pool(name="xload", bufs=8) as xpool,
            tc.tile_pool(name="pexp", bufs=8) as ppool,
            tc.tile_pool(name="bcast", bufs=3) as bpool,
            tc.tile_pool(name="ostage", bufs=4) as opool,
            tc.tile_pool(name="lrow", bufs=2, space="DRAM") as dpool,
            tc.tile_pool(name="psum", bufs=2, space="PSUM") as psum,
            tc.tile_pool(name="psum_av", bufs=1, space="PSUM") as psum_av,
        ):

            def emit_x(b):
                for st in range(N_ST):
                    g = b * N_ST + st
                    x_t = xpool.tile([P, DC, ST], F16, tag="xt", name="x_t")
                    nc.sync.dma_start(x_t[:], xt_r[:, :, g * ST : (g + 1) * ST])
                    xts[b][st] = x_t

            def emit_proj(b, st, w_sb, b_sb, dst):
                # One 512-wide projection chain: dst[:, st] = W @ X^T + bias
                sl = slice(st * ST, (st + 1) * ST)
                ps = psum.tile([P, ST], F32, tag="mm", name="ps")
                for dc in range(DC):
                    nc.tensor.matmul(
                        ps[:], w_sb[:, dc], xts[b][st][:, dc],
                        start=(dc == 0), stop=(dc == DC - 1),
                    )
                nc.vector.tensor_scalar_add(dst[:, sl], ps[:], b_sb[:])

            def emit_trans(b, chunks):
                # PE-transpose batch b's V chunks into V_ext.
                for tch in chunks:
                    tp = psum.tile([P, ST], F16, tag="mm", name="tp")
                    nc.tensor.transpose(
                        tp[:, 0:P], vt[b][:, tch * P : (tch + 1) * P], ident[:]
                    )
                    nc.vector.tensor_copy(vx[b][:, tch, 0:E], tp[:, 0:E])
                    nc.vector.tensor_copy(
                        vx[b][:, tch, E + 1 : 2 * E + 1], tp[:, E : 2 * E]
                    )

            def emit_outproj(b, st):
                for c in range(ST // P):
                    zsl = slice(st * ST + c * P, st * ST + (c + 1) * P)
                    rows = slice(b * S + st * ST + c * P, b * S + st * ST + (c + 1) * P)
                    for oh in range(D // 512):
                        po = psum.tile([P, ST], F32, tag="mm", name="po")
                        nc.tensor.matmul(
                            po[:], z[b][:, zsl], wo_sb[:, oh * 512 : (oh + 1) * 512],
                            start=True, stop=True,
                        )
                        osb = opool.tile([P, 512], F16, tag="osb", name="osb")
                        nc.vector.tensor_copy(osb[:], po[:])
                        nc.sync.dma_start(outp[rows, oh * 512 : (oh + 1) * 512], osb[:])

            pending = [None]

            def emit_attn(b, st, hooks):
                ssl = slice(st * ST, (st + 1) * ST)
                av = psum_av.tile([P, 2, ST], F32, tag="av", name="av")
                av0 = av[:, 0]
                av1 = av[:, 1]
                for t in range(TCH):
                    tsl = slice(t * P, (t + 1) * P)
                    sc = psum.tile([P, 2, ST], F32, tag="sc", name="sc")
                    nc.tensor.matmul(
                        sc[:, 0], kt[b][0:E, tsl], qt[b][0:E, ssl],
                        start=True, stop=True,
                    )
                    nc.tensor.matmul(
                        sc[:, 1], kt[b][E : 2 * E, tsl], qt[b][E : 2 * E, ssl],
                        start=True, stop=True,
                    )
                    pt = ppool.tile([P, 2, ST], F16, tag="pt", name="pt")
                    nc.scalar.activation(pt[:], sc[:], EXP, scale=0.125)
                    for fn in hooks.get(t, ()):
                        fn()
                    if t == 8 and pending[0] is not None:
                        emit_outproj(*pending[0])
                        pending[0] = None
                    nc.tensor.matmul(
                        av0[0 : E + 1], vx[b][:, t, 0 : E + 1], pt[:, 0],
                        start=(t == 0), stop=(t == TCH - 1),
                    )
                    nc.tensor.matmul(
                        av1[0 : E + 1], vx[b][:, t, E + 1 : 2 * E + 2], pt[:, 1],
                        start=(t == 0), stop=(t == TCH - 1),
                    )
                # Drain: unnormalized copy out of PSUM (frees the av banks),
                # reciprocal of the fused row-sums, broadcast via DRAM bounce.
                # All of this is off the critical path: the out-projection that
                # consumes z is deferred into the next s-tile's t-loop.
                nc.vector.tensor_copy(z[b][0:E, ssl], av0[0:E])
                nc.vector.tensor_copy(z[b][E : 2 * E, ssl], av1[0:E])
                lr = bpool.tile([1, 2, ST], F32, tag="lr", name="lr")
                nc.vector.tensor_copy(lr[0:1], av[E : E + 1, :, :])
                nc.vector.reciprocal_approx_fast(out=lr[0:1], in_=lr[0:1])
                lrow = dpool.tile([2, ST], F32, tag="lrow", name="lrow")
                nc.sync.dma_start(
                    bass.AP(tensor=lrow.tensor, offset=lrow.offset,
                            ap=[[0, 1]] + list(lrow.ap)),
                    lr[0:1, :, :],
                )
                bc = bpool.tile([P, ST], F32, tag="bc", name="bc")
                nc.sync.dma_start(
                    bc[0:E],
                    bass.AP(tensor=lrow.tensor, offset=lrow.offset,
                            ap=[[0, E]] + list(lrow[0, :].ap)),
                )
                nc.sync.dma_start(
                    bc[E : 2 * E],
                    bass.AP(tensor=lrow.tensor, offset=lrow.offset + ST,
                            ap=[[0, E]] + list(lrow[1, :].ap)),
                )
                nc.vector.tensor_mul(z[b][0:E, ssl], z[b][0:E, ssl], bc[0:E])
                nc.vector.tensor_mul(
                    z[b][E : 2 * E, ssl], z[b][E : 2 * E, ssl], bc[E : 2 * E]
                )
                pending[0] = (b, st)

            # ---- hook tables: all projection / transpose / DMA work for the
            # next batch is emitted inside the current batch's t-loops so the
            # Tile scheduler paces it into the PE gaps under the exp stream.
            def hooks_for(b, st):
                hk = {}

                def add(t, fn):
                    hk.setdefault(t, []).append(fn)

                if st + 1 < N_ST:
                    # this batch's next-s-tile Q projection, well before needed
                    add(2, lambda: emit_proj(b, st + 1, wq_sb, bq_sb, qt[b]))
                nb = b + 1
                if nb < B:
                    if st == 0:
                        add(4, lambda: emit_x(nb))
                        add(6, lambda: emit_proj(nb, 0, wk_sb, bk_sb, kt[nb]))
                        add(10, lambda: emit_proj(nb, 1, wk_sb, bk_sb, kt[nb]))
                        add(14, lambda: emit_proj(nb, 2, wk_sb, bk_sb, kt[nb]))
                    elif st == 1:
                        add(4, lambda: emit_proj(nb, 3, wk_sb, bk_sb, kt[nb]))
                        add(6, lambda: emit_proj(nb, 0, wv_sb, bv_sb, vt[nb]))
                        add(10, lambda: emit_trans(nb, range(0, 4)))
                        add(14, lambda: emit_proj(nb, 1, wv_sb, bv_sb, vt[nb]))
                    elif st == 2:
                        add(4, lambda: emit_trans(nb, range(4, 8)))
                        add(6, lambda: emit_proj(nb, 2, wv_sb, bv_sb, vt[nb]))
                        add(10, lambda: emit_trans(nb, range(8, 12)))
                        add(14, lambda: emit_proj(nb, 3, wv_sb, bv_sb, vt[nb]))
                    else:
                        add(4, lambda: emit_trans(nb, range(12, 16)))
                        add(6, lambda: emit_proj(nb, 0, wq_sb, bq_sb, qt[nb]))
                return hk

            # ---- batch 0 head: K first so the exp stream starts ASAP ----
            emit_x(0)
            emit_proj(0, 0, wk_sb, bk_sb, kt[0])
            emit_proj(0, 1, wk_sb, bk_sb, kt[0])
            emit_proj(0, 0, wq_sb, bq_sb, qt[0])
            emit_proj(0, 0, wv_sb, bv_sb, vt[0])

            for st in range(N_ST):
                hk = hooks_for(0, st)
                if st == 0:
                    # batch 0's own remaining projections. NOTE: producers
                    # must be EMITTED before their consumers (program order
                    # defines dependencies) — trans(c) before av(t=c).
                    hk.setdefault(0, []).insert(0, lambda: emit_trans(0, range(0, 4)))
                    hk.setdefault(0, []).append(
                        lambda: emit_proj(0, 1, wv_sb, bv_sb, vt[0]))
                    hk.setdefault(1, []).append(
                        lambda: emit_proj(0, 2, wk_sb, bk_sb, kt[0]))
                    hk.setdefault(4, []).append(lambda: emit_trans(0, range(4, 8)))
                    hk.setdefault(4, []).append(
                        lambda: emit_proj(0, 2, wv_sb, bv_sb, vt[0]))
                    hk.setdefault(5, []).append(
                        lambda: emit_proj(0, 3, wk_sb, bk_sb, kt[0]))
                    hk.setdefault(8, []).insert(0, lambda: emit_trans(0, range(8, 12)))
                    hk.setdefault(8, []).append(
                        lambda: emit_proj(0, 3, wv_sb, bv_sb, vt[0]))
                    hk.setdefault(12, []).append(lambda: emit_trans(0, range(12, 16)))
                emit_attn(0, st, hk)

            for b in range(1, B):
                for st in range(N_ST):
                    emit_attn(b, st, hooks_for(b, st))

            emit_outproj(*pending[0])
    nc.finalize()
    return nc


_NC_CACHE = None


def _get_module():
    global _NC_CACHE
    if _NC_CACHE is None:
        _NC_CACHE = build_module()
    return _NC_CACHE


def prepare_in_maps(inputs):
    x = np.ascontiguousarray(np.asarray(inputs["input_matrix"], np.float32))
    wq = np.asarray(inputs["Wq"], np.float32)
    wk = np.asarray(inputs["Wk"], np.float32)
    wv = np.asarray(inputs["Wv"], np.float32)
    bq = np.asarray(inputs["bq"], np.float32)
    bk = np.asarray(inputs["bk"], np.float32)
    bv = np.asarray(inputs["bv"], np.float32)
    wo = np.asarray(inputs["Wo"], np.float32)

    xt = np.ascontiguousarray(x.reshape(BS, D).T.astype(np.float16))  # [D, BS]
    in_maps = []
    for c in range(NCORES):
        hs = slice(HPC * c, HPC * (c + 1))
        m = {
            "xt": xt,
            "wq_t": np.ascontiguousarray(wq[hs].transpose(2, 0, 1).reshape(D, EC).astype(np.float16)),
            "wk_t": np.ascontiguousarray(wk[hs].transpose(2, 0, 1).reshape(D, EC).astype(np.float16)),
            "wv_t": np.ascontiguousarray(wv[hs].transpose(2, 0, 1).reshape(D, EC).astype(np.float16)),
            "bq": np.ascontiguousarray(bq[hs].reshape(EC, 1)),
            "bk": np.ascontiguousarray(bk[hs].reshape(EC, 1)),
            "bv": np.ascontiguousarray(bv[hs].reshape(EC, 1)),
            "wo_t": np.ascontiguousarray(wo[:, EC * c : EC * (c + 1)].T.astype(np.float16)),
        }
        in_maps.append(m)
    return in_maps


def finish(results, inputs):
    bo = np.asarray(inputs["bo"], np.float32)
    acc = results[0]["out_p"].astype(np.float32)
    for r in results[1:]:
        acc += r["out_p"].astype(np.float32)
    out = (acc + bo).astype(np.float32)
    return out.reshape(B, S, D)


def kernel(**inputs):
    nc = _get_module()
    in_maps = prepare_in_maps(inputs)
    res = run_bass_kernel_spmd(nc, in_maps, core_ids=list(range(NCORES)))
    return finish(res.results, inputs)


if __name__ == "__main__":
    import reference

    inputs = {k: np.asarray(v) for k, v in reference.setup_inputs().items()}
    out = kernel(**inputs)
    print(out.shape, out.dtype)
